# revision 7
# baseline (speedup 1.0000x reference)
"""MLA (multi-head latent attention) Trainium2 Bass kernel.

Sharding: 8 cores = batch(2) x head-groups(4 heads each). No cross-core
collectives: both latent projections are computed full-sequence on every
core (redundant within a batch group, but cheaper and far more robust on
HW than the 4-core latent AllGather, which measured ~300-400us on real
hardware vs 67us in the cost model, and desyncs under pipelining).

Structure:
- Phase A streams x^T once per 512-seq chunk and runs three accumulation
  passes (c_q, c_kv, k_rope) against it; rmsnorm stats via a ones-column
  matmul, rstd broadcast via a tiny fp32 PE matmul (keeps the Pool queue
  free); then q/k/v up-projections.
- Phase B attention runs in "o transposed" orientation: o[d, q]
  accumulates with V tiles stationary and exp-scores streaming, removing
  all PE transposes. Softmax denominators: zacc[j%%128, q] accumulated on
  gpsimd as es tiles are produced, folded to z[1, q] by a ones-matmul,
  reciprocal broadcast, and applied to o with one DVE multiply per group
  (software-pipelined one group behind attention).
- All matmuls bf16 with fp32 PSUM; causal-masked tiles stream only live
  columns.
"""

import sys
import numpy as np
import ml_dtypes

for _p in ("/opt/trn_rl_repo", "/root/.axon_site/_ro/trn_rl_repo"):
    if _p not in sys.path:
        sys.path.append(_p)

BF16 = ml_dtypes.bfloat16

D_MODEL = 2048
SEQ = 2048
BATCH = 2
N_HEADS = 16
D_HEAD = 128
D_KV = 512
D_ROPE = 64
ROPE_BASE = 10000.0
EPS = 1e-5
H_LOC = 4          # heads per core
N_CORES = 8

_BUILD_CACHE = {}


def build_program(reps: int = 1):
    if reps in _BUILD_CACHE:
        return _BUILD_CACHE[reps]

    import concourse.bass as bass  # noqa: F401
    import concourse.mybir as mybir
    from concourse import bacc
    from concourse.tile import TileContext
    from contextlib import ExitStack

    f32 = mybir.dt.float32
    bf16 = mybir.dt.bfloat16
    AF = mybir.ActivationFunctionType
    OP = mybir.AluOpType

    nc = bacc.Bacc(num_devices=8)

    xT = nc.declare_dram_parameter("xT", [D_MODEL, SEQ], bf16, isOutput=False)
    wdq = nc.declare_dram_parameter("wdq", [D_MODEL, D_KV], bf16, isOutput=False)
    wdkv = nc.declare_dram_parameter("wdkv", [D_MODEL, D_KV], bf16, isOutput=False)
    wq = nc.declare_dram_parameter("wq", [D_KV, H_LOC * 128], bf16, isOutput=False)
    wuk2 = nc.declare_dram_parameter("wuk2", [D_KV, 2 * 128], bf16, isOutput=False)
    wkr2 = nc.declare_dram_parameter("wkr2", [D_MODEL, 2 * 128], bf16, isOutput=False)
    wuv = nc.declare_dram_parameter("wuv", [D_KV, H_LOC * 128], bf16, isOutput=False)
    wout = nc.declare_dram_parameter("wout", [H_LOC * 128, D_MODEL], bf16, isOutput=False)
    mult = nc.declare_dram_parameter("mult", [128, 2 * SEQ], bf16, isOutput=False)
    masks = nc.declare_dram_parameter("masks", [128, 4 * 512], bf16, isOutput=False)
    y = nc.declare_dram_parameter("y", [SEQ, D_MODEL], bf16, isOutput=True)

    SCALE = 1.0 / float(np.sqrt(np.float32(D_HEAD)))
    NKT = D_MODEL // 128    # 16 k-tiles over d_model
    NLT = D_KV // 128       # 4  k-tiles over latent
    NSN = SEQ // 512        # 4  sequence chunks
    NQB = SEQ // 512        # 4  q blocks

    with TileContext(nc) as tc, ExitStack() as top:
        pp = top.enter_context(tc.tile_pool(name="persist", bufs=1))
        qt_sb = pp.tile([128, H_LOC * SEQ], bf16, tag="qt")
        kt_sb = pp.tile([128, H_LOC * SEQ], bf16, tag="kt")
        v_sb = pp.tile([128, (SEQ // 128) * 512], bf16, tag="v")

        for _rep in range(reps):
            # ================= Phase A =================
            with ExitStack() as pa:
                pA = pa.enter_context(tc.tile_pool(name="phA", bufs=1))
                pCf = pa.enter_context(tc.tile_pool(name="pCf", bufs=5))
                pCs = pa.enter_context(tc.tile_pool(name="pCs", bufs=9))
                pX = pa.enter_context(tc.tile_pool(name="pX", bufs=20))
                pR = pa.enter_context(tc.tile_pool(name="pR", bufs=4))
                psA = pa.enter_context(tc.tile_pool(name="psA", bufs=6, space="PSUM"))
                psS = pa.enter_context(tc.tile_pool(name="psS", bufs=1, space="PSUM"))
                psBC = pa.enter_context(tc.tile_pool(name="psBC", bufs=1, space="PSUM"))

                cq_sb = pA.tile([128, NLT * SEQ], bf16, tag="cq")
                ckv_sb = pA.tile([128, NLT * SEQ], bf16, tag="ckv")
                wdq_sb = pA.tile([128, NKT * D_KV], bf16, tag="wdq")
                wdkv_sb = pA.tile([128, NKT * D_KV], bf16, tag="wdkv")
                wq_sb = pA.tile([128, NLT * 512], bf16, tag="wq")
                wuk2_sb = pA.tile([128, NLT * 256], bf16, tag="wuk2")
                wkr2_sb = pA.tile([128, NKT * 256], bf16, tag="wkr2")
                wuv_sb = pA.tile([128, NLT * 512], bf16, tag="wuv")
                mult_sb = pA.tile([128, 2 * SEQ], bf16, tag="mult")
                ones_sb = pA.tile([128, 1], bf16, tag="ones")
                onesr_sb = pA.tile([1, 128], bf16, tag="onesr")
                eps_sb = pA.tile([1, 1], f32, tag="eps")

                for kt in range(NKT):
                    nc.gpsimd.dma_start(out=wdq_sb[:, kt * D_KV:(kt + 1) * D_KV],
                                        in_=wdq[kt * 128:(kt + 1) * 128, :])
                for kt in range(NKT):
                    nc.gpsimd.dma_start(out=wdkv_sb[:, kt * D_KV:(kt + 1) * D_KV],
                                        in_=wdkv[kt * 128:(kt + 1) * 128, :])
                for kt in range(NKT):
                    nc.gpsimd.dma_start(out=wkr2_sb[:, kt * 256:(kt + 1) * 256],
                                        in_=wkr2[kt * 128:(kt + 1) * 128, :])
                for lt in range(NLT):
                    nc.gpsimd.dma_start(out=wq_sb[:, lt * 512:(lt + 1) * 512],
                                        in_=wq[lt * 128:(lt + 1) * 128, :])
                    nc.gpsimd.dma_start(out=wuk2_sb[:, lt * 256:(lt + 1) * 256],
                                        in_=wuk2[lt * 128:(lt + 1) * 128, :])
                    nc.gpsimd.dma_start(out=wuv_sb[:, lt * 512:(lt + 1) * 512],
                                        in_=wuv[lt * 128:(lt + 1) * 128, :])
                nc.vector.memset(ones_sb[:], 1.0)
                nc.vector.memset(onesr_sb[:], 1.0)
                nc.vector.memset(eps_sb[:], EPS)

                def rmsnorm_chain(cps_l, sq_l, cn_dst):
                    """From 4 accumulated latent PSUM tiles: stats -> rstd ->
                    normalized bf16 tiles written to cn_dst(lt) APs."""
                    ss = psS.tile([1, 512], f32, tag="stat")
                    for lt in range(NLT):
                        nc.tensor.matmul(ss[:], ones_sb[:], sq_l[lt][:],
                                         start=(lt == 0), stop=(lt == NLT - 1))
                    lnv = pR.tile([1, 512], f32, tag="ln")
                    nc.scalar.activation(lnv[:], ss[:], AF.Ln, scale=1.0 / D_KV,
                                         bias=eps_sb[:])
                    rstd = pR.tile([1, 512], f32, tag="rstd")
                    nc.scalar.activation(rstd[:], lnv[:], AF.Exp, scale=-0.5)
                    bstd = psBC.tile([128, 512], f32, tag="bstd")
                    nc.tensor.matmul(bstd[:], onesr_sb[:], rstd[:], start=True, stop=True)
                    for lt in range(NLT):
                        nc.vector.tensor_tensor(cn_dst(lt), cps_l[lt][:], bstd[:], OP.mult)

                # ---- stage 2: both latents (full seq) + kr, one xT stream ----
                # Per seq chunk sn: load the 16 x-tiles once, then three
                # accumulation passes (c_q, c_kv, kr) reuse them from SBUF.
                for sn in range(NSN):
                    xts = []
                    for kt in range(NKT):
                        xt = pX.tile([128, 512], bf16, tag="xs", name=f"xs{sn}{kt}")
                        nc.sync.dma_start(
                            out=xt[:],
                            in_=xT[kt * 128:(kt + 1) * 128, sn * 512:(sn + 1) * 512])
                        xts.append(xt)
                    for cname, wd_sb, dst_sb in (("q", wdq_sb, cq_sb),
                                                 ("kv", wdkv_sb, ckv_sb)):
                        c_cps, c_sq = [], []
                        c_ps = [psA.tile([128, 512], f32, tag="mm",
                                         name=f"c{cname}{sn}{lt}")
                                for lt in range(NLT)]
                        for kt in range(NKT):
                            for lt in range(NLT):
                                nc.tensor.matmul(
                                    c_ps[lt][:],
                                    wd_sb[:, kt * D_KV + lt * 128: kt * D_KV + (lt + 1) * 128],
                                    xts[kt][:],
                                    start=(kt == 0), stop=(kt == NKT - 1))
                        for lt in range(NLT):
                            cps = pCf.tile([128, 512], f32, tag="cpre",
                                           name=f"c{cname}c{sn}{lt}")
                            nc.vector.tensor_copy(cps[:], c_ps[lt][:])
                            sq = pCs.tile([128, 512], bf16, tag="sq",
                                          name=f"c{cname}s{sn}{lt}")
                            nc.vector.tensor_tensor(sq[:], cps[:], cps[:], OP.mult)
                            c_cps.append(cps)
                            c_sq.append(sq)
                        rmsnorm_chain(
                            c_cps, c_sq,
                            lambda lt: dst_sb[:, lt * SEQ + sn * 512: lt * SEQ + (sn + 1) * 512])
                    # kr pass
                    kr_ps = [psA.tile([128, 512], f32, tag="mm", name=f"kr{sn}{p}")
                             for p in range(2)]
                    for kt in range(NKT):
                        for p in range(2):
                            nc.tensor.matmul(
                                kr_ps[p][:],
                                wkr2_sb[:, kt * 256 + p * 128: kt * 256 + (p + 1) * 128],
                                xts[kt][:],
                                start=(kt == 0), stop=(kt == NKT - 1))
                    for p in range(2):
                        h0, h1 = 2 * p, 2 * p + 1
                        kp = kr_ps[p]
                        m0 = mult_sb[:, 0 * SEQ + sn * 512: 0 * SEQ + (sn + 1) * 512]
                        m1 = mult_sb[:, 1 * SEQ + sn * 512: 1 * SEQ + (sn + 1) * 512]
                        k0 = kt_sb[:, h0 * SEQ + sn * 512: h0 * SEQ + (sn + 1) * 512]
                        k1 = kt_sb[:, h1 * SEQ + sn * 512: h1 * SEQ + (sn + 1) * 512]
                        nc.vector.tensor_tensor(k0[64:128, :], kp[64:128, :], m0[64:128, :], OP.mult)
                        nc.vector.tensor_tensor(k1[0:64, :], kp[0:64, :], m1[0:64, :], OP.mult)

                # ---- stage 4: up-projections ----
                for sn in range(NSN):
                    def cnq(lt):
                        return cq_sb[:, lt * SEQ + sn * 512: lt * SEQ + (sn + 1) * 512]
                    def cnkv(lt):
                        return ckv_sb[:, lt * SEQ + sn * 512: lt * SEQ + (sn + 1) * 512]
                    for hl in range(H_LOC):
                        qp = psA.tile([128, 512], f32, tag="mm", name=f"q{sn}{hl}")
                        for lt in range(NLT):
                            nc.tensor.matmul(
                                qp[:],
                                wq_sb[:, lt * 512 + hl * 128: lt * 512 + (hl + 1) * 128],
                                cnq(lt),
                                start=(lt == 0), stop=(lt == NLT - 1))
                        nc.vector.tensor_tensor(
                            qt_sb[:, hl * SEQ + sn * 512: hl * SEQ + (sn + 1) * 512],
                            qp[:],
                            mult_sb[:, (hl % 2) * SEQ + sn * 512: (hl % 2) * SEQ + (sn + 1) * 512],
                            OP.mult)
                    for p in range(2):
                        h0, h1 = 2 * p, 2 * p + 1
                        up = psA.tile([128, 512], f32, tag="mm", name=f"k{sn}{p}")
                        for lt in range(NLT):
                            nc.tensor.matmul(
                                up[:],
                                wuk2_sb[:, lt * 256 + p * 128: lt * 256 + (p + 1) * 128],
                                cnkv(lt),
                                start=(lt == 0), stop=(lt == NLT - 1))
                        m0 = mult_sb[:, 0 * SEQ + sn * 512: 0 * SEQ + (sn + 1) * 512]
                        m1 = mult_sb[:, 1 * SEQ + sn * 512: 1 * SEQ + (sn + 1) * 512]
                        k0 = kt_sb[:, h0 * SEQ + sn * 512: h0 * SEQ + (sn + 1) * 512]
                        k1 = kt_sb[:, h1 * SEQ + sn * 512: h1 * SEQ + (sn + 1) * 512]
                        nc.vector.tensor_tensor(k0[0:64, :], up[0:64, :], m0[0:64, :], OP.mult)
                        nc.vector.tensor_tensor(k1[64:128, :], up[64:128, :], m1[64:128, :], OP.mult)
                    for st in range(4):
                        s_tile = sn * 4 + st
                        vp = psA.tile([128, 512], f32, tag="mm", name=f"v{sn}{st}")
                        for lt in range(NLT):
                            nc.tensor.matmul(
                                vp[:],
                                cnkv(lt)[:, st * 128:(st + 1) * 128],
                                wuv_sb[:, lt * 512:(lt + 1) * 512],
                                start=(lt == 0), stop=(lt == NLT - 1))
                        nc.vector.tensor_copy(
                            v_sb[:, s_tile * 512:(s_tile + 1) * 512], vp[:])

            # ================= Phase B =================
            with ExitStack() as pb:
                pB = pb.enter_context(tc.tile_pool(name="phB", bufs=1))
                pEs = pb.enter_context(tc.tile_pool(name="pEs", bufs=8))
                pZa = pb.enter_context(tc.tile_pool(name="pZa", bufs=3))
                pZr = pb.enter_context(tc.tile_pool(name="pZr", bufs=2))
                pO = pb.enter_context(tc.tile_pool(name="pO", bufs=6))
                pYs = pb.enter_context(tc.tile_pool(name="pYs", bufs=4))
                psSc = pb.enter_context(tc.tile_pool(name="psSc", bufs=3, space="PSUM"))
                psO = pb.enter_context(tc.tile_pool(name="psO", bufs=2, space="PSUM"))
                psZ = pb.enter_context(tc.tile_pool(name="psZ", bufs=1, space="PSUM"))
                psY = pb.enter_context(tc.tile_pool(name="psY", bufs=2, space="PSUM"))

                masks_sb = pB.tile([128, 4 * 512], bf16, tag="masks")
                wout_sb = pB.tile([128, H_LOC * D_MODEL], bf16, tag="wout")
                onesb_sb = pB.tile([128, 1], bf16, tag="ones2")
                nc.sync.dma_start(out=masks_sb[:], in_=masks[:, :])
                for f in range(H_LOC):
                    nc.sync.dma_start(out=wout_sb[:, f * D_MODEL:(f + 1) * D_MODEL],
                                      in_=wout[f * 128:(f + 1) * 128, :])
                nc.vector.memset(onesb_sb[:], 1.0)

                # one-group software pipeline: normalization of group g is
                # emitted after group g+1's attention loop.
                pending = []
                o_norm = {}

                def flush_pending():
                    for (qb0, hl0, o_ps, zacc) in pending:
                        zbf = pEs.tile([128, 512], bf16, tag="zbf", name=f"zbf{qb0}{hl0}")
                        nc.vector.tensor_copy(zbf[:], zacc[:])
                        zp = psZ.tile([1, 512], f32, tag="z", name=f"z{qb0}{hl0}")
                        nc.tensor.matmul(zp[:], onesb_sb[:], zbf[:], start=True, stop=True)
                        zr = pZr.tile([1, 512], f32, tag="zr", name=f"zr{qb0}{hl0}")
                        nc.vector.reciprocal(zr[:], zp[:])
                        zb = pZr.tile([128, 512], f32, tag="zb", name=f"zb{qb0}{hl0}")
                        nc.gpsimd.partition_broadcast(zb[:], zr[:])
                        onrm = pO.tile([128, 512], bf16, tag="on", name=f"on{qb0}{hl0}")
                        nc.vector.tensor_tensor(onrm[:], o_ps[:], zb[:], OP.mult)
                        o_norm[(qb0, hl0)] = onrm
                    pending.clear()

                def emit_yproj(qb0):
                    for qs in range(4):
                        row0 = qb0 * 512 + qs * 128
                        for ncol in range(4):
                            yp = psY.tile([128, 512], f32, tag="y", name=f"y{qb0}{qs}{ncol}")
                            for f in range(H_LOC):
                                nc.tensor.matmul(
                                    yp[:],
                                    o_norm[(qb0, f)][:, qs * 128:(qs + 1) * 128],
                                    wout_sb[:, f * D_MODEL + ncol * 512: f * D_MODEL + (ncol + 1) * 512],
                                    start=(f == 0), stop=(f == H_LOC - 1))
                            ys = pYs.tile([128, 512], bf16, tag="ysb")
                            nc.vector.tensor_copy(ys[:], yp[:])
                            nc.sync.dma_start(
                                out=y[row0:row0 + 128, ncol * 512:(ncol + 1) * 512],
                                in_=ys[:])

                for qb in range(NQB):
                    njt = (qb + 1) * 4
                    for hl in range(H_LOC):
                        o_ps = psO.tile([128, 512], f32, tag="o", name=f"o{qb}{hl}")
                        zacc = pZa.tile([128, 512], f32, tag="za", name=f"za{qb}{hl}")
                        for jt in range(njt):
                            kd = jt - qb * 4
                            c0 = max(kd, 0) * 128
                            sp = psSc.tile([128, 512], f32, tag="s", name=f"s{qb}{hl}{jt}")
                            nc.tensor.matmul(
                                sp[:, c0:],
                                kt_sb[:, hl * SEQ + jt * 128: hl * SEQ + (jt + 1) * 128],
                                qt_sb[:, hl * SEQ + qb * 512 + c0: hl * SEQ + (qb + 1) * 512],
                                start=True, stop=True)
                            es = pEs.tile([128, 512], bf16, tag="es", name=f"e{qb}{hl}{jt}")
                            nc.scalar.activation(es[:, c0:], sp[:, c0:], AF.Exp, scale=SCALE)
                            if kd >= 0:
                                nc.vector.tensor_tensor(
                                    es[:, c0:], es[:, c0:],
                                    masks_sb[:, kd * 512 + c0:(kd + 1) * 512], OP.mult)
                            if jt == 0:
                                nc.gpsimd.tensor_copy(zacc[:], es[:])
                            else:
                                nc.gpsimd.tensor_tensor(
                                    zacc[:, c0:], zacc[:, c0:], es[:, c0:], OP.add)
                            nc.tensor.matmul(
                                o_ps[:, c0:],
                                v_sb[:, jt * 512 + hl * 128: jt * 512 + (hl + 1) * 128],
                                es[:, c0:],
                                start=(jt == 0), stop=(jt == njt - 1),
                                skip_group_check=True)
                        flush_pending()
                        pending.append((qb, hl, o_ps, zacc))
                        if hl == H_LOC - 1 and qb > 0:
                            emit_yproj(qb - 1)
                    # qb's own last group normalization flushes during qb+1,
                    # except for the final qb which flushes below.
                flush_pending()
                emit_yproj(NQB - 1)

    nc.finalize()
    _BUILD_CACHE[reps] = nc
    return nc


def _rope_mult():
    half = D_ROPE // 2
    theta = 1.0 / (ROPE_BASE ** (np.arange(0, D_HEAD, 2, dtype=np.float32) / D_HEAD))
    idx = np.arange(SEQ, dtype=np.float32)[:, None] * theta[None, :]
    r = np.tile(np.cos(idx[:, :half]), (1, 2)) + np.tile(np.sin(idx[:, :half]), (1, 2))
    return np.ascontiguousarray(r.T).astype(np.float32)  # [64, SEQ]


def make_inputs(x, W_dq, W_uq, W_dkv, W_uk, W_uv, W_qr, W_kr, g_q, g_kv, W_out, b_out):
    rT = _rope_mult()
    mult = np.empty((128, 2 * SEQ), np.float32)
    mult[0:64, 0:SEQ] = 1.0
    mult[64:128, 0:SEQ] = rT
    mult[0:64, SEQ:] = rT
    mult[64:128, SEQ:] = 1.0
    mult = mult.astype(BF16)

    masks = np.zeros((128, 4 * 512), np.float32)
    jl = np.arange(128)[:, None]
    ql = np.arange(512)[None, :]
    for k in range(4):
        masks[:, k * 512:(k + 1) * 512] = (ql >= 128 * k + jl)
    masks = masks.astype(BF16)

    gq = g_q.astype(np.float32)[:, None]
    gkv = g_kv.astype(np.float32)[:, None]
    Wuq_g = W_uq * gq
    Wqr_g = W_qr * gq
    Wuk_g = W_uk * gkv
    Wuv_g = W_uv * gkv

    in_maps = []
    for core in range(N_CORES):
        b = core // 4
        g = core % 4
        heads = [4 * g + i for i in range(H_LOC)]

        xb = np.ascontiguousarray(x[b].T).astype(BF16)  # [d_model, seq]

        wq_pack = np.empty((D_KV, H_LOC * 128), np.float32)
        for hl, h in enumerate(heads):
            a = Wuq_g[:, h * 64:(h + 1) * 64]
            r = Wqr_g[:, h * 64:(h + 1) * 64]
            blk = np.concatenate([a, r], axis=1) if hl % 2 == 0 else np.concatenate([r, a], axis=1)
            wq_pack[:, hl * 128:(hl + 1) * 128] = blk

        wuk2 = np.empty((D_KV, 256), np.float32)
        wkr2 = np.empty((D_MODEL, 256), np.float32)
        for p in range(2):
            h0, h1 = heads[2 * p], heads[2 * p + 1]
            wuk2[:, p * 128: p * 128 + 64] = Wuk_g[:, h0 * 64:(h0 + 1) * 64]
            wuk2[:, p * 128 + 64: p * 128 + 128] = Wuk_g[:, h1 * 64:(h1 + 1) * 64]
            wkr2[:, p * 128: p * 128 + 64] = W_kr[:, h1 * 64:(h1 + 1) * 64]
            wkr2[:, p * 128 + 64: p * 128 + 128] = W_kr[:, h0 * 64:(h0 + 1) * 64]

        wuv_pack = np.concatenate(
            [Wuv_g[:, h * 128:(h + 1) * 128] for h in heads], axis=1)
        wout_pack = np.concatenate(
            [W_out[h * 128:(h + 1) * 128, :] for h in heads], axis=0)

        in_maps.append({
            "xT": xb,
            "wdq": W_dq.astype(BF16),
            "wdkv": W_dkv.astype(BF16),
            "wq": wq_pack.astype(BF16),
            "wuk2": wuk2.astype(BF16),
            "wkr2": wkr2.astype(BF16),
            "wuv": wuv_pack.astype(BF16),
            "wout": wout_pack.astype(BF16),
            "mult": mult,
            "masks": masks,
        })
    return in_maps


def kernel(**inputs):
    inputs = {k: np.asarray(v) for k, v in inputs.items()}
    in_maps = make_inputs(
        inputs["x"], inputs["W_dq"], inputs["W_uq"], inputs["W_dkv"],
        inputs["W_uk"], inputs["W_uv"], inputs["W_qr"], inputs["W_kr"],
        inputs["g_q"], inputs["g_kv"], inputs["W_out"], inputs["b_out"])

    nc = build_program(reps=1)
    from concourse.bass_utils import run_bass_kernel_spmd
    res = run_bass_kernel_spmd(nc, in_maps, list(range(N_CORES)))

    b_out = inputs["b_out"].astype(np.float32)
    out = np.zeros((BATCH, SEQ, D_MODEL), np.float32)
    for core in range(N_CORES):
        out[core // 4] += res.results[core]["y"].astype(np.float32)
    out += b_out[None, None, :]
    return out


# revision 8
# speedup vs baseline: 1.1201x; 1.1201x over previous
"""MLA (multi-head latent attention) Trainium2 Bass kernel.

Sharding: 8 cores = batch(2) x head-groups(4 heads each). No cross-core
collectives: both latent projections are computed full-sequence on every
core (redundant within a batch group, but cheaper and far more robust on
HW than the 4-core latent AllGather, which measured ~300-400us on real
hardware vs 67us in the cost model, and desyncs under pipelining).

Structure:
- Phase A streams x^T once per 512-seq chunk and runs three accumulation
  passes (c_q, c_kv, k_rope) against it; rmsnorm stats via a ones-column
  matmul, rstd broadcast via a tiny fp32 PE matmul (keeps the Pool queue
  free); then q/k/v up-projections.
- Phase B attention runs in "o transposed" orientation: o[d, q]
  accumulates with V tiles stationary and exp-scores streaming, removing
  all PE transposes. Softmax denominators: zacc[j%%128, q] accumulated on
  gpsimd as es tiles are produced, folded to z[1, q] by a ones-matmul,
  reciprocal broadcast, and applied to o with one DVE multiply per group
  (software-pipelined one group behind attention).
- All matmuls bf16 with fp32 PSUM; causal-masked tiles stream only live
  columns.
"""

import sys
import numpy as np
import ml_dtypes

for _p in ("/opt/trn_rl_repo", "/root/.axon_site/_ro/trn_rl_repo"):
    if _p not in sys.path:
        sys.path.append(_p)

BF16 = ml_dtypes.bfloat16

D_MODEL = 2048
SEQ = 2048
BATCH = 2
N_HEADS = 16
D_HEAD = 128
D_KV = 512
D_ROPE = 64
ROPE_BASE = 10000.0
EPS = 1e-5
H_LOC = 4          # heads per core
N_CORES = 8

_BUILD_CACHE = {}


def build_program(reps: int = 1):
    if reps in _BUILD_CACHE:
        return _BUILD_CACHE[reps]

    import concourse.bass as bass  # noqa: F401
    import concourse.mybir as mybir
    from concourse import bacc
    from concourse.tile import TileContext
    from contextlib import ExitStack

    f32 = mybir.dt.float32
    bf16 = mybir.dt.bfloat16
    AF = mybir.ActivationFunctionType
    OP = mybir.AluOpType

    nc = bacc.Bacc(num_devices=8)

    xT = nc.declare_dram_parameter("xT", [D_MODEL, SEQ], bf16, isOutput=False)
    wdq = nc.declare_dram_parameter("wdq", [D_MODEL, D_KV], bf16, isOutput=False)
    wdkv = nc.declare_dram_parameter("wdkv", [D_MODEL, D_KV], bf16, isOutput=False)
    wq = nc.declare_dram_parameter("wq", [D_KV, H_LOC * 128], bf16, isOutput=False)
    wuk2 = nc.declare_dram_parameter("wuk2", [D_KV, 2 * 128], bf16, isOutput=False)
    wkr2 = nc.declare_dram_parameter("wkr2", [D_MODEL, 2 * 128], bf16, isOutput=False)
    wuv = nc.declare_dram_parameter("wuv", [D_KV, H_LOC * 128], bf16, isOutput=False)
    wout = nc.declare_dram_parameter("wout", [H_LOC * 128, D_MODEL], bf16, isOutput=False)
    mult = nc.declare_dram_parameter("mult", [128, 2 * SEQ], bf16, isOutput=False)
    masks = nc.declare_dram_parameter("masks", [128, 4 * 512], bf16, isOutput=False)
    y = nc.declare_dram_parameter("y", [SEQ, D_MODEL], bf16, isOutput=True)

    SCALE = 1.0 / float(np.sqrt(np.float32(D_HEAD)))
    NKT = D_MODEL // 128    # 16 k-tiles over d_model
    NLT = D_KV // 128       # 4  k-tiles over latent
    NSN = SEQ // 512        # 4  sequence chunks
    NQB = SEQ // 512        # 4  q blocks

    with TileContext(nc) as tc, ExitStack() as top:
        pp = top.enter_context(tc.tile_pool(name="persist", bufs=1))
        qt_sb = pp.tile([128, H_LOC * SEQ], bf16, tag="qt")
        kt_sb = pp.tile([128, H_LOC * SEQ], bf16, tag="kt")
        v_sb = pp.tile([128, (SEQ // 128) * 512], bf16, tag="v")

        for _rep in range(reps):
            # ================= Phase A =================
            with ExitStack() as pa:
                pA = pa.enter_context(tc.tile_pool(name="phA", bufs=1))
                pCf = pa.enter_context(tc.tile_pool(name="pCf", bufs=5))
                pCs = pa.enter_context(tc.tile_pool(name="pCs", bufs=9))
                pX = pa.enter_context(tc.tile_pool(name="pX", bufs=20))
                pR = pa.enter_context(tc.tile_pool(name="pR", bufs=4))
                psA = pa.enter_context(tc.tile_pool(name="psA", bufs=6, space="PSUM"))
                psS = pa.enter_context(tc.tile_pool(name="psS", bufs=1, space="PSUM"))
                psBC = pa.enter_context(tc.tile_pool(name="psBC", bufs=1, space="PSUM"))

                cq_sb = pA.tile([128, NLT * SEQ], bf16, tag="cq")
                ckv_sb = pA.tile([128, NLT * SEQ], bf16, tag="ckv")
                wdq_sb = pA.tile([128, NKT * D_KV], bf16, tag="wdq")
                wdkv_sb = pA.tile([128, NKT * D_KV], bf16, tag="wdkv")
                wq_sb = pA.tile([128, NLT * 512], bf16, tag="wq")
                wuk2_sb = pA.tile([128, NLT * 256], bf16, tag="wuk2")
                wkr2_sb = pA.tile([128, NKT * 256], bf16, tag="wkr2")
                wuv_sb = pA.tile([128, NLT * 512], bf16, tag="wuv")
                mult_sb = pA.tile([128, 2 * SEQ], bf16, tag="mult")
                ones_sb = pA.tile([128, 1], bf16, tag="ones")
                onesr_sb = pA.tile([1, 128], bf16, tag="onesr")
                eps_sb = pA.tile([1, 1], f32, tag="eps")

                for kt in range(NKT):
                    nc.gpsimd.dma_start(out=wdq_sb[:, kt * D_KV:(kt + 1) * D_KV],
                                        in_=wdq[kt * 128:(kt + 1) * 128, :])
                for kt in range(NKT):
                    nc.gpsimd.dma_start(out=wdkv_sb[:, kt * D_KV:(kt + 1) * D_KV],
                                        in_=wdkv[kt * 128:(kt + 1) * 128, :])
                for kt in range(NKT):
                    nc.gpsimd.dma_start(out=wkr2_sb[:, kt * 256:(kt + 1) * 256],
                                        in_=wkr2[kt * 128:(kt + 1) * 128, :])
                for lt in range(NLT):
                    nc.gpsimd.dma_start(out=wq_sb[:, lt * 512:(lt + 1) * 512],
                                        in_=wq[lt * 128:(lt + 1) * 128, :])
                    nc.gpsimd.dma_start(out=wuk2_sb[:, lt * 256:(lt + 1) * 256],
                                        in_=wuk2[lt * 128:(lt + 1) * 128, :])
                    nc.gpsimd.dma_start(out=wuv_sb[:, lt * 512:(lt + 1) * 512],
                                        in_=wuv[lt * 128:(lt + 1) * 128, :])
                nc.vector.memset(ones_sb[:], 1.0)
                nc.vector.memset(onesr_sb[:], 1.0)
                nc.vector.memset(eps_sb[:], EPS)

                def rmsnorm_chain(cps_l, sq_l, cn_dst):
                    """From 4 accumulated latent PSUM tiles: stats -> rstd ->
                    normalized bf16 tiles written to cn_dst(lt) APs."""
                    ss = psS.tile([1, 512], f32, tag="stat")
                    for lt in range(NLT):
                        nc.tensor.matmul(ss[:], ones_sb[:], sq_l[lt][:],
                                         start=(lt == 0), stop=(lt == NLT - 1))
                    lnv = pR.tile([1, 512], f32, tag="ln")
                    nc.scalar.activation(lnv[:], ss[:], AF.Ln, scale=1.0 / D_KV,
                                         bias=eps_sb[:])
                    rstd = pR.tile([1, 512], f32, tag="rstd")
                    nc.scalar.activation(rstd[:], lnv[:], AF.Exp, scale=-0.5)
                    bstd = psBC.tile([128, 512], f32, tag="bstd")
                    nc.tensor.matmul(bstd[:], onesr_sb[:], rstd[:], start=True, stop=True)
                    for lt in range(NLT):
                        nc.vector.tensor_tensor(cn_dst(lt), cps_l[lt][:], bstd[:], OP.mult)

                # ---- stage 2: both latents (full seq) + kr, one xT stream ----
                # Per seq chunk sn: load the 16 x-tiles once, then three
                # accumulation passes (c_q, c_kv, kr) reuse them from SBUF.
                for sn in range(NSN):
                    xts = []
                    for kt in range(NKT):
                        xt = pX.tile([128, 512], bf16, tag="xs", name=f"xs{sn}{kt}")
                        nc.sync.dma_start(
                            out=xt[:],
                            in_=xT[kt * 128:(kt + 1) * 128, sn * 512:(sn + 1) * 512])
                        xts.append(xt)
                    for cname, wd_sb, dst_sb in (("q", wdq_sb, cq_sb),
                                                 ("kv", wdkv_sb, ckv_sb)):
                        c_cps, c_sq = [], []
                        c_ps = [psA.tile([128, 512], f32, tag="mm",
                                         name=f"c{cname}{sn}{lt}")
                                for lt in range(NLT)]
                        for kt in range(NKT):
                            for lt in range(NLT):
                                nc.tensor.matmul(
                                    c_ps[lt][:],
                                    wd_sb[:, kt * D_KV + lt * 128: kt * D_KV + (lt + 1) * 128],
                                    xts[kt][:],
                                    start=(kt == 0), stop=(kt == NKT - 1))
                        for lt in range(NLT):
                            cps = pCf.tile([128, 512], f32, tag="cpre",
                                           name=f"c{cname}c{sn}{lt}")
                            nc.vector.tensor_copy(cps[:], c_ps[lt][:])
                            sq = pCs.tile([128, 512], bf16, tag="sq",
                                          name=f"c{cname}s{sn}{lt}")
                            nc.vector.tensor_tensor(sq[:], cps[:], cps[:], OP.mult)
                            c_cps.append(cps)
                            c_sq.append(sq)
                        rmsnorm_chain(
                            c_cps, c_sq,
                            lambda lt: dst_sb[:, lt * SEQ + sn * 512: lt * SEQ + (sn + 1) * 512])
                    # kr pass
                    kr_ps = [psA.tile([128, 512], f32, tag="mm", name=f"kr{sn}{p}")
                             for p in range(2)]
                    for kt in range(NKT):
                        for p in range(2):
                            nc.tensor.matmul(
                                kr_ps[p][:],
                                wkr2_sb[:, kt * 256 + p * 128: kt * 256 + (p + 1) * 128],
                                xts[kt][:],
                                start=(kt == 0), stop=(kt == NKT - 1))
                    for p in range(2):
                        h0, h1 = 2 * p, 2 * p + 1
                        kp = kr_ps[p]
                        m0 = mult_sb[:, 0 * SEQ + sn * 512: 0 * SEQ + (sn + 1) * 512]
                        m1 = mult_sb[:, 1 * SEQ + sn * 512: 1 * SEQ + (sn + 1) * 512]
                        k0 = kt_sb[:, h0 * SEQ + sn * 512: h0 * SEQ + (sn + 1) * 512]
                        k1 = kt_sb[:, h1 * SEQ + sn * 512: h1 * SEQ + (sn + 1) * 512]
                        nc.vector.tensor_tensor(k0[64:128, :], kp[64:128, :], m0[64:128, :], OP.mult)
                        nc.vector.tensor_tensor(k1[0:64, :], kp[0:64, :], m1[0:64, :], OP.mult)

                # ---- stage 4: up-projections ----
                for sn in range(NSN):
                    def cnq(lt):
                        return cq_sb[:, lt * SEQ + sn * 512: lt * SEQ + (sn + 1) * 512]
                    def cnkv(lt):
                        return ckv_sb[:, lt * SEQ + sn * 512: lt * SEQ + (sn + 1) * 512]
                    for hl in range(H_LOC):
                        qp = psA.tile([128, 512], f32, tag="mm", name=f"q{sn}{hl}")
                        for lt in range(NLT):
                            nc.tensor.matmul(
                                qp[:],
                                wq_sb[:, lt * 512 + hl * 128: lt * 512 + (hl + 1) * 128],
                                cnq(lt),
                                start=(lt == 0), stop=(lt == NLT - 1))
                        nc.vector.tensor_tensor(
                            qt_sb[:, hl * SEQ + sn * 512: hl * SEQ + (sn + 1) * 512],
                            qp[:],
                            mult_sb[:, (hl % 2) * SEQ + sn * 512: (hl % 2) * SEQ + (sn + 1) * 512],
                            OP.mult)
                    for p in range(2):
                        h0, h1 = 2 * p, 2 * p + 1
                        up = psA.tile([128, 512], f32, tag="mm", name=f"k{sn}{p}")
                        for lt in range(NLT):
                            nc.tensor.matmul(
                                up[:],
                                wuk2_sb[:, lt * 256 + p * 128: lt * 256 + (p + 1) * 128],
                                cnkv(lt),
                                start=(lt == 0), stop=(lt == NLT - 1))
                        m0 = mult_sb[:, 0 * SEQ + sn * 512: 0 * SEQ + (sn + 1) * 512]
                        m1 = mult_sb[:, 1 * SEQ + sn * 512: 1 * SEQ + (sn + 1) * 512]
                        k0 = kt_sb[:, h0 * SEQ + sn * 512: h0 * SEQ + (sn + 1) * 512]
                        k1 = kt_sb[:, h1 * SEQ + sn * 512: h1 * SEQ + (sn + 1) * 512]
                        nc.vector.tensor_tensor(k0[0:64, :], up[0:64, :], m0[0:64, :], OP.mult)
                        nc.vector.tensor_tensor(k1[64:128, :], up[64:128, :], m1[64:128, :], OP.mult)
                    for st in range(4):
                        s_tile = sn * 4 + st
                        vp = psA.tile([128, 512], f32, tag="mm", name=f"v{sn}{st}")
                        for lt in range(NLT):
                            nc.tensor.matmul(
                                vp[:],
                                cnkv(lt)[:, st * 128:(st + 1) * 128],
                                wuv_sb[:, lt * 512:(lt + 1) * 512],
                                start=(lt == 0), stop=(lt == NLT - 1))
                        nc.vector.tensor_copy(
                            v_sb[:, s_tile * 512:(s_tile + 1) * 512], vp[:])

            # ================= Phase B =================
            with ExitStack() as pb:
                pB = pb.enter_context(tc.tile_pool(name="phB", bufs=1))
                pEs = pb.enter_context(tc.tile_pool(name="pEs", bufs=8))
                pZa = pb.enter_context(tc.tile_pool(name="pZa", bufs=3))
                pZr = pb.enter_context(tc.tile_pool(name="pZr", bufs=2))
                pO = pb.enter_context(tc.tile_pool(name="pO", bufs=6))
                pYs = pb.enter_context(tc.tile_pool(name="pYs", bufs=4))
                psSc = pb.enter_context(tc.tile_pool(name="psSc", bufs=2, space="PSUM"))
                psO = pb.enter_context(tc.tile_pool(name="psO", bufs=3, space="PSUM"))
                psZ = pb.enter_context(tc.tile_pool(name="psZ", bufs=1, space="PSUM"))
                psY = pb.enter_context(tc.tile_pool(name="psY", bufs=2, space="PSUM"))

                masks_sb = pB.tile([128, 4 * 512], bf16, tag="masks")
                wout_sb = pB.tile([128, H_LOC * D_MODEL], bf16, tag="wout")
                onesb_sb = pB.tile([128, 1], bf16, tag="ones2")
                nc.sync.dma_start(out=masks_sb[:], in_=masks[:, :])
                for f in range(H_LOC):
                    nc.sync.dma_start(out=wout_sb[:, f * D_MODEL:(f + 1) * D_MODEL],
                                      in_=wout[f * 128:(f + 1) * 128, :])
                nc.vector.memset(onesb_sb[:], 1.0)

                # two-group software pipeline: normalization of group g is
                # emitted after group g+2's attention loop, so the Pool-queue
                # broadcast never waits on the z-matmul round trip.
                pending = []
                o_norm = {}

                def flush_pending(keep=0):
                    while len(pending) > keep:
                        (qb0, hl0, o_ps, zacc) = pending.pop(0)
                        zbf = pEs.tile([128, 512], bf16, tag="zbf", name=f"zbf{qb0}{hl0}")
                        nc.vector.tensor_copy(zbf[:], zacc[:])
                        zp = psZ.tile([1, 512], f32, tag="z", name=f"z{qb0}{hl0}")
                        nc.tensor.matmul(zp[:], onesb_sb[:], zbf[:], start=True, stop=True)
                        zr = pZr.tile([1, 512], f32, tag="zr", name=f"zr{qb0}{hl0}")
                        nc.vector.reciprocal(zr[:], zp[:])
                        zb = pZr.tile([128, 512], f32, tag="zb", name=f"zb{qb0}{hl0}")
                        nc.gpsimd.partition_broadcast(zb[:], zr[:])
                        onrm = pO.tile([128, 512], bf16, tag="on", name=f"on{qb0}{hl0}")
                        nc.vector.tensor_tensor(onrm[:], o_ps[:], zb[:], OP.mult)
                        o_norm[(qb0, hl0)] = onrm

                def emit_yproj(qb0):
                    for qs in range(4):
                        row0 = qb0 * 512 + qs * 128
                        for ncol in range(4):
                            yp = psY.tile([128, 512], f32, tag="y", name=f"y{qb0}{qs}{ncol}")
                            for f in range(H_LOC):
                                nc.tensor.matmul(
                                    yp[:],
                                    o_norm[(qb0, f)][:, qs * 128:(qs + 1) * 128],
                                    wout_sb[:, f * D_MODEL + ncol * 512: f * D_MODEL + (ncol + 1) * 512],
                                    start=(f == 0), stop=(f == H_LOC - 1))
                            ys = pYs.tile([128, 512], bf16, tag="ysb")
                            nc.vector.tensor_copy(ys[:], yp[:])
                            nc.sync.dma_start(
                                out=y[row0:row0 + 128, ncol * 512:(ncol + 1) * 512],
                                in_=ys[:])

                for qb in range(NQB):
                    njt = (qb + 1) * 4
                    for hl in range(H_LOC):
                        o_ps = psO.tile([128, 512], f32, tag="o", name=f"o{qb}{hl}")
                        zacc = pZa.tile([128, 512], f32, tag="za", name=f"za{qb}{hl}")
                        for jt in range(njt):
                            kd = jt - qb * 4
                            c0 = max(kd, 0) * 128
                            sp = psSc.tile([128, 512], f32, tag="s", name=f"s{qb}{hl}{jt}")
                            nc.tensor.matmul(
                                sp[:, c0:],
                                kt_sb[:, hl * SEQ + jt * 128: hl * SEQ + (jt + 1) * 128],
                                qt_sb[:, hl * SEQ + qb * 512 + c0: hl * SEQ + (qb + 1) * 512],
                                start=True, stop=True)
                            es = pEs.tile([128, 512], bf16, tag="es", name=f"e{qb}{hl}{jt}")
                            nc.scalar.activation(es[:, c0:], sp[:, c0:], AF.Exp, scale=SCALE)
                            if kd >= 0:
                                nc.vector.tensor_tensor(
                                    es[:, c0:], es[:, c0:],
                                    masks_sb[:, kd * 512 + c0:(kd + 1) * 512], OP.mult)
                            if jt == 0:
                                nc.gpsimd.tensor_copy(zacc[:], es[:])
                            else:
                                nc.gpsimd.tensor_tensor(
                                    zacc[:, c0:], zacc[:, c0:], es[:, c0:], OP.add)
                            nc.tensor.matmul(
                                o_ps[:, c0:],
                                v_sb[:, jt * 512 + hl * 128: jt * 512 + (hl + 1) * 128],
                                es[:, c0:],
                                start=(jt == 0), stop=(jt == njt - 1),
                                skip_group_check=True)
                        flush_pending(keep=1)
                        pending.append((qb, hl, o_ps, zacc))
                        if hl == H_LOC - 1 and qb > 0:
                            emit_yproj(qb - 1)
                    # qb's own last group normalization flushes during qb+1,
                    # except for the final qb which flushes below.
                flush_pending()
                emit_yproj(NQB - 1)

    nc.finalize()
    _BUILD_CACHE[reps] = nc
    return nc


def _rope_mult():
    half = D_ROPE // 2
    theta = 1.0 / (ROPE_BASE ** (np.arange(0, D_HEAD, 2, dtype=np.float32) / D_HEAD))
    idx = np.arange(SEQ, dtype=np.float32)[:, None] * theta[None, :]
    r = np.tile(np.cos(idx[:, :half]), (1, 2)) + np.tile(np.sin(idx[:, :half]), (1, 2))
    return np.ascontiguousarray(r.T).astype(np.float32)  # [64, SEQ]


def make_inputs(x, W_dq, W_uq, W_dkv, W_uk, W_uv, W_qr, W_kr, g_q, g_kv, W_out, b_out):
    rT = _rope_mult()
    mult = np.empty((128, 2 * SEQ), np.float32)
    mult[0:64, 0:SEQ] = 1.0
    mult[64:128, 0:SEQ] = rT
    mult[0:64, SEQ:] = rT
    mult[64:128, SEQ:] = 1.0
    mult = mult.astype(BF16)

    masks = np.zeros((128, 4 * 512), np.float32)
    jl = np.arange(128)[:, None]
    ql = np.arange(512)[None, :]
    for k in range(4):
        masks[:, k * 512:(k + 1) * 512] = (ql >= 128 * k + jl)
    masks = masks.astype(BF16)

    gq = g_q.astype(np.float32)[:, None]
    gkv = g_kv.astype(np.float32)[:, None]
    Wuq_g = W_uq * gq
    Wqr_g = W_qr * gq
    Wuk_g = W_uk * gkv
    Wuv_g = W_uv * gkv

    in_maps = []
    for core in range(N_CORES):
        b = core // 4
        g = core % 4
        heads = [4 * g + i for i in range(H_LOC)]

        xb = np.ascontiguousarray(x[b].T).astype(BF16)  # [d_model, seq]

        wq_pack = np.empty((D_KV, H_LOC * 128), np.float32)
        for hl, h in enumerate(heads):
            a = Wuq_g[:, h * 64:(h + 1) * 64]
            r = Wqr_g[:, h * 64:(h + 1) * 64]
            blk = np.concatenate([a, r], axis=1) if hl % 2 == 0 else np.concatenate([r, a], axis=1)
            wq_pack[:, hl * 128:(hl + 1) * 128] = blk

        wuk2 = np.empty((D_KV, 256), np.float32)
        wkr2 = np.empty((D_MODEL, 256), np.float32)
        for p in range(2):
            h0, h1 = heads[2 * p], heads[2 * p + 1]
            wuk2[:, p * 128: p * 128 + 64] = Wuk_g[:, h0 * 64:(h0 + 1) * 64]
            wuk2[:, p * 128 + 64: p * 128 + 128] = Wuk_g[:, h1 * 64:(h1 + 1) * 64]
            wkr2[:, p * 128: p * 128 + 64] = W_kr[:, h1 * 64:(h1 + 1) * 64]
            wkr2[:, p * 128 + 64: p * 128 + 128] = W_kr[:, h0 * 64:(h0 + 1) * 64]

        wuv_pack = np.concatenate(
            [Wuv_g[:, h * 128:(h + 1) * 128] for h in heads], axis=1)
        wout_pack = np.concatenate(
            [W_out[h * 128:(h + 1) * 128, :] for h in heads], axis=0)

        in_maps.append({
            "xT": xb,
            "wdq": W_dq.astype(BF16),
            "wdkv": W_dkv.astype(BF16),
            "wq": wq_pack.astype(BF16),
            "wuk2": wuk2.astype(BF16),
            "wkr2": wkr2.astype(BF16),
            "wuv": wuv_pack.astype(BF16),
            "wout": wout_pack.astype(BF16),
            "mult": mult,
            "masks": masks,
        })
    return in_maps


def kernel(**inputs):
    inputs = {k: np.asarray(v) for k, v in inputs.items()}
    in_maps = make_inputs(
        inputs["x"], inputs["W_dq"], inputs["W_uq"], inputs["W_dkv"],
        inputs["W_uk"], inputs["W_uv"], inputs["W_qr"], inputs["W_kr"],
        inputs["g_q"], inputs["g_kv"], inputs["W_out"], inputs["b_out"])

    nc = build_program(reps=1)
    from concourse.bass_utils import run_bass_kernel_spmd
    res = run_bass_kernel_spmd(nc, in_maps, list(range(N_CORES)))

    b_out = inputs["b_out"].astype(np.float32)
    out = np.zeros((BATCH, SEQ, D_MODEL), np.float32)
    for core in range(N_CORES):
        out[core // 4] += res.results[core]["y"].astype(np.float32)
    out += b_out[None, None, :]
    return out
